# revision 1
# baseline (speedup 1.0000x reference)
"""Multi-head causal attention (B=2, N=2048, D=1024, H=16) on 8 TRN2 NeuronCores.

Sharding: data-parallel over batch (2) x tensor-parallel over head groups (4),
so each core handles one batch element and 4 heads (256 of the 1024 hidden
channels). Wq/Wk/Wv are column-sharded, Wo row-sharded; each core emits a
partial output [2048, 1024] that the host sums over the 4 head groups.

Per-core dataflow (all matmuls bf16 with fp32 PSUM accumulation):
  xT (pre-transposed on host)      [1024, 2048]
  Q^T = Wq_c^T x^T, K^T likewise   [256, 2048]   (head h at partition 64*(h%2), m-tile h//2)
  V   = x Wv_c                     [2048, 256]   stored per seq-tile with an
                                   appended ones column per head (the ones row
                                   of the U matmul accumulates the softmax
                                   denominator alongside the context)
  S^T               [128k, 1024]   both heads of a pair packed per k-tile: the
                                   even head (partitions 0-63 of K^T/Q^T, PE row
                                   strips 0-1) writes cols 0-511, the odd head
                                   (partitions 64-127, strips 2-3) cols 512-1023
                                   -> one exp() ACTIVATE covers both heads
  expS = exp(S^T/8); causal masking multiplies the 4 diagonal-crossing tiles
  by precomputed 0/1 masks (exp never overflows: |S/8| < ~4 at this scale)
  U = V_aug^T expS accumulated over k-tiles; the denominator row is partition
  64 (even head) / 0 (odd head); ctx^T = U[data] * bcast(1/r) where 1/r uses
  reciprocal_approx_fast and the partition-broadcast goes through a DRAM
  bounce (step-0 partition APs are only legal for DRAM sources, and the
  gpsimd partition_broadcast ucode is broken on this runtime).
  Y = ctx^T^T Wo_c                 [2048, 1024] fp32 partial out, emitted per
                                   q-chunk so the output projection overlaps
                                   the next chunk's attention.
"""

import sys

sys.path.insert(0, "/opt/trn_rl_repo")

import numpy as np
import ml_dtypes

import concourse.bass as bass
import concourse.bacc as bacc
import concourse.mybir as mybir
from concourse.tile import TileContext
from concourse.bass_utils import run_bass_kernel_spmd

BF16 = mybir.dt.bfloat16
F32 = mybir.dt.float32

B, N, D, H = 2, 2048, 1024, 16
HD = 64          # head dim
HPC = 4          # heads per core
DH = HPC * HD    # 256 hidden channels per core
NCORES = 8
KT = D // 128    # 8 contraction tiles over D
ST = N // 128    # 16 seq tiles
QC = N // 512    # 4 q-chunks of 512

# v_sb per-seq-tile column layout: for each head pair, an "even" block
# [V(64) | ones(1)] (matmul M=65 -> U partitions 0..64, denom at 64) and an
# "odd" block [ones(1) | zeros(63) | V(64)] (M=128 -> U partitions 64..127
# hold data, denom at partition 0, zeros keep partitions 1..63 inert).
V_BLK = {0: (0, 65), 1: (65, 193), 2: (193, 258), 3: (258, 386)}
V_COLS = 386
V_DATA_OFF = {0: 0, 1: 129, 2: 193, 3: 322}


def _y_tiles_for_iteration(qc):
    # Y seq-tiles emitted during attention iteration qc: chunk qc-1's tiles
    # while qc < QC runs, plus chunk QC-1's own tiles at the end.
    tiles = []
    if qc > 0:
        tiles += list(range(4 * (qc - 1), 4 * qc))
    if qc == QC - 1:
        tiles += list(range(4 * qc, 4 * (qc + 1)))
    return tiles


def _build_nc(debug: bool = False) -> bass.Bass:
    nc = bacc.Bacc()
    xT = nc.declare_dram_parameter("xT", [D, N], BF16, isOutput=False)
    wq = nc.declare_dram_parameter("wq", [D, DH], BF16, isOutput=False)
    wk = nc.declare_dram_parameter("wk", [D, DH], BF16, isOutput=False)
    wv = nc.declare_dram_parameter("wv", [D, DH], BF16, isOutput=False)
    wo = nc.declare_dram_parameter("wo", [DH, D], BF16, isOutput=False)
    y = nc.declare_dram_parameter("y", [N, D], F32, isOutput=True)
    if debug:
        dbg = {
            "d_qT": nc.declare_dram_parameter("d_qT", [128, 2 * N], F32, isOutput=True),
            "d_kT": nc.declare_dram_parameter("d_kT", [128, 2 * N], F32, isOutput=True),
            "d_v": nc.declare_dram_parameter("d_v", [128, ST * V_COLS], F32, isOutput=True),
            "d_ctxT": nc.declare_dram_parameter("d_ctxT", [128, 2 * N], F32, isOutput=True),
        }

    xT_r = xT.rearrange("(t p) n -> t p n", p=128)
    wq_r = wq.rearrange("(t p) m -> t p m", p=128)
    wk_r = wk.rearrange("(t p) m -> t p m", p=128)
    wv_r = wv.rearrange("(t p) m -> t p m", p=128)
    wo_r = wo.rearrange("(t p) m -> t p m", p=128)
    y_r = y.rearrange("(t p) m -> t p m", p=128)

    with TileContext(nc) as tc:
        with (
            tc.tile_pool(name="const", bufs=1) as cpool,
            tc.tile_pool(name="io", bufs=3) as io_pool,
            tc.tile_pool(name="exps", bufs=6) as exp_pool,
            tc.tile_pool(name="small", bufs=4) as small_pool,
            tc.tile_pool(name="ps_big", bufs=2, space="PSUM") as ps_big,
            tc.tile_pool(name="ps_u", bufs=2, space="PSUM") as ps_u_pool,
            tc.tile_pool(name="drams", bufs=1, space="DRAM") as dram_pool,
        ):
            rscr = dram_pool.tile([16, 512], F32)
            xT_sb = cpool.tile([128, KT, N], BF16)
            wq_sb = cpool.tile([128, KT, DH], BF16)
            wk_sb = cpool.tile([128, KT, DH], BF16)
            wv_sb = cpool.tile([128, KT, DH], BF16)
            wo_sb = cpool.tile([128, 2, D], BF16)
            qT_sb = cpool.tile([128, 2, N], BF16)
            kT_sb = cpool.tile([128, 2, N], BF16)
            v_sb = cpool.tile([128, ST, V_COLS], BF16)
            ctxT_sb = cpool.tile([128, 2, N], BF16)
            masks_sb = cpool.tile([128, 4, 1024], BF16)

            # xT and wq gate the first matmuls — land them first
            for t in range(KT):
                nc.sync.dma_start(out=xT_sb[:, t, :], in_=xT_r[t])
                nc.sync.dma_start(out=wq_sb[:, t, :], in_=wq_r[t])
            for t in range(KT):
                nc.sync.dma_start(out=wk_sb[:, t, :], in_=wk_r[t])
                nc.sync.dma_start(out=wv_sb[:, t, :], in_=wv_r[t])
            for t in range(2):
                nc.sync.dma_start(out=wo_sb[:, t, :], in_=wo_r[t])

            # Causal masks for the 4 diagonal-crossing k-tiles of a q-chunk:
            # keep (1.0) where dq >= dk + 128*i, replicated in both halves so
            # one [128, 1024] multiply masks both packed heads.
            for i in range(4):
                nc.vector.memset(masks_sb[:, i, :], 1.0)
                m2 = masks_sb[:, i, :].rearrange("p (h q) -> p h q", q=512)
                nc.gpsimd.affine_select(
                    out=m2,
                    in_=m2,
                    compare_op=mybir.AluOpType.is_ge,
                    fill=0.0,
                    base=-(128 * i),
                    pattern=[[0, 2], [1, 512]],
                    channel_multiplier=-1,
                )

            # ones / zeros scaffolding of the V blocks (all seq tiles at once)
            nc.vector.memset(v_sb[:, :, 66:129], 0.0)
            nc.vector.memset(v_sb[:, :, 259:322], 0.0)
            for col in (64, 65, 257, 258):
                nc.vector.memset(v_sb[:, :, col : col + 1], 1.0)

            # ---- Q^T / K^T projections: [256, 2048] each, 2 q-chunks per PSUM ----
            for w_sb, dst in ((wq_sb, qT_sb), (wk_sb, kT_sb)):
                for mt in range(2):
                    for qcp in range(QC // 2):
                        ps = ps_big.tile([128, 1024], F32, tag="big", name="ps")
                        for kt in range(KT):
                            for half in range(2):
                                qc = 2 * qcp + half
                                nc.tensor.matmul(
                                    ps[:, 512 * half : 512 * (half + 1)],
                                    lhsT=w_sb[:, kt, 128 * mt : 128 * (mt + 1)],
                                    rhs=xT_sb[:, kt, 512 * qc : 512 * (qc + 1)],
                                    start=(kt == 0),
                                    stop=(kt == KT - 1),
                                )
                        nc.vector.tensor_copy(
                            dst[:, mt, 1024 * qcp : 1024 * (qcp + 1)], ps
                        )

            # ---- V = x @ Wv_c, stored per seq tile with ones columns ----
            for st in range(ST):
                ps = ps_big.tile([128, 1024], F32, tag="big", name="ps")
                psv = ps[:, 0:DH]
                for kt in range(KT):
                    nc.tensor.matmul(
                        psv,
                        lhsT=xT_sb[:, kt, 128 * st : 128 * (st + 1)],
                        rhs=wv_sb[:, kt, :],
                        start=(kt == 0),
                        stop=(kt == KT - 1),
                    )
                ps_h = ps.rearrange("p (h d) -> p h d", d=HD)
                # even heads 0,2 -> offsets 0,193; odd heads 1,3 -> 129,322
                ev = bass.AP(
                    tensor=v_sb.tensor,
                    offset=v_sb[:, st, 0:1].offset,
                    ap=[v_sb.ap[0], [193, 2], [1, HD]],
                )
                od = bass.AP(
                    tensor=v_sb.tensor,
                    offset=v_sb[:, st, 129:130].offset,
                    ap=[v_sb.ap[0], [193, 2], [1, HD]],
                )
                in_ev = bass.AP(
                    tensor=ps.tensor,
                    offset=ps_h[:, 0, :].offset,
                    ap=[ps.ap[0], [2 * HD, 2], [1, HD]],
                )
                in_od = bass.AP(
                    tensor=ps.tensor,
                    offset=ps_h[:, 1, :].offset,
                    ap=[ps.ap[0], [2 * HD, 2], [1, HD]],
                )
                nc.vector.tensor_copy(ev, in_ev)
                nc.vector.tensor_copy(od, in_od)

            # ---- attention + output projection, interleaved per q-chunk ----
            for qc in range(QC):
                nkt = 4 * (qc + 1)
                for mt in range(2):
                    ps_u = {
                        0: ps_u_pool.tile([128, 512], F32, tag="ue", name="ue"),
                        1: ps_u_pool.tile([128, 512], F32, tag="uo", name="uo"),
                    }
                    def _pv(ex_prev, kt_prev):
                        for parity in (0, 1):
                            head = 2 * mt + parity
                            blo, bhi = V_BLK[head]
                            nc.tensor.matmul(
                                ps_u[parity][0 : bhi - blo, :],
                                lhsT=v_sb[:, kt_prev, blo:bhi],
                                rhs=ex_prev[:, 512 * parity : 512 * (parity + 1)],
                                start=(kt_prev == 0),
                                stop=(kt_prev == nkt - 1),
                            )

                    # PV lags one k-tile behind S so the in-order PE queue
                    # never has a PV (waiting on exp) ahead of ready S matmuls
                    prev = None
                    for kt in range(nkt):
                        # S^T for both heads of the pair into one 2-bank tile
                        ps_s = ps_big.tile([128, 1024], F32, tag="big", name="ps_s")
                        for parity in (0, 1):
                            pofs = 64 * parity
                            nc.tensor.matmul(
                                ps_s[:, 512 * parity : 512 * (parity + 1)],
                                lhsT=kT_sb[
                                    pofs : pofs + 64, mt, 128 * kt : 128 * (kt + 1)
                                ],
                                rhs=qT_sb[
                                    pofs : pofs + 64, mt, 512 * qc : 512 * (qc + 1)
                                ],
                                start=True,
                                stop=True,
                            )
                        ex = exp_pool.tile([128, 1024], BF16)
                        nc.scalar.activation(
                            ex,
                            ps_s,
                            mybir.ActivationFunctionType.Exp,
                            scale=1.0 / np.sqrt(HD),
                        )
                        di = kt - 4 * qc
                        if di >= 0:
                            nc.vector.tensor_mul(ex, ex, masks_sb[:, di, :])
                        if prev is not None:
                            _pv(*prev)
                        prev = (ex, kt)
                    _pv(*prev)
                    for parity in (0, 1):
                        # reciprocal_approx_fast (custom DVE ucode) only works
                        # on APs based at partition 0 on this runtime: the odd
                        # head (denom at partition 0) takes 1/r before the
                        # DRAM-bounce broadcast; the even head (denom at
                        # partition 64) broadcasts raw r to partitions 0-63
                        # first and takes the reciprocal there.
                        pofs = 64 * parity
                        r_part = 64 if parity == 0 else 0
                        data_lo = 0 if parity == 0 else 64
                        u = ps_u[parity]
                        ridx = (mt * QC + qc) * 2 + parity
                        rinv = small_pool.tile([128, 512], F32, tag="rinv")
                        if parity == 1:
                            nc.vector.reciprocal_approx_fast(
                                out=rinv[0:1, :], in_=u[0:1, :]
                            )
                            src = rinv[0:1, :]
                        else:
                            nc.vector.tensor_copy(rinv[64:65, :], u[64:65, :])
                            src = rinv[64:65, :]
                        nc.sync.dma_start(out=rscr[ridx : ridx + 1, :], in_=src)
                        rb = small_pool.tile([128, 512], F32, tag="rb")
                        bsrc = bass.AP(
                            tensor=rscr.tensor,
                            offset=rscr[ridx : ridx + 1, :].offset,
                            ap=[[0, 64]] + list(rscr[ridx : ridx + 1, :].ap[1:]),
                        )
                        nc.gpsimd.dma_start(out=rb[pofs : pofs + 64, :], in_=bsrc)
                        if parity == 0:
                            nc.vector.reciprocal_approx_fast(
                                out=rb[0:64, :], in_=rb[0:64, :]
                            )
                        nc.vector.tensor_mul(
                            ctxT_sb[pofs : pofs + 64, mt, 512 * qc : 512 * (qc + 1)],
                            u[data_lo : data_lo + 64, :],
                            rb[pofs : pofs + 64, :],
                        )

                # output projection runs one q-chunk behind the attention so
                # the in-order PE queue never stalls on the normalize chain
                # (U -> reciprocal -> DRAM-bounce broadcast -> ctx multiply).
                for st in _y_tiles_for_iteration(qc):
                    ps = ps_big.tile([128, 1024], F32, tag="big", name="ps")
                    for half in range(2):
                        for kt2 in range(2):
                            nc.tensor.matmul(
                                ps[:, 512 * half : 512 * (half + 1)],
                                lhsT=ctxT_sb[:, kt2, 128 * st : 128 * (st + 1)],
                                rhs=wo_sb[:, kt2, 512 * half : 512 * (half + 1)],
                                start=(kt2 == 0),
                                stop=(kt2 == 1),
                            )
                    ysb = io_pool.tile([128, 1024], F32)
                    nc.vector.tensor_copy(ysb, ps)
                    nc.sync.dma_start(out=y_r[st], in_=ysb)

            if debug:
                for nm, sb in (
                    ("d_qT", qT_sb),
                    ("d_kT", kT_sb),
                    ("d_v", v_sb),
                    ("d_ctxT", ctxT_sb),
                ):
                    flat = sb.rearrange("p a b -> p (a b)")
                    w = flat.shape[1]
                    for off in range(0, w, 512):
                        wid = min(512, w - off)
                        tmp2 = io_pool.tile([128, 1024], F32, tag="dtmp", name="dtmp")
                        nc.vector.tensor_copy(tmp2[:, 0:wid], flat[:, off : off + wid])
                        nc.sync.dma_start(
                            out=dbg[nm][:, off : off + wid], in_=tmp2[:, 0:wid]
                        )
    nc.finalize()
    return nc


_NC = None


def _get_nc():
    global _NC
    if _NC is None:
        _NC = _build_nc()
    return _NC


def kernel(x, Wq, Wk, Wv, Wo):
    x = np.asarray(x, dtype=np.float32)
    bf = ml_dtypes.bfloat16
    in_maps = []
    for c in range(NCORES):
        b, g = divmod(c, 4)
        sl = slice(g * DH, (g + 1) * DH)
        in_maps.append(
            {
                "xT": np.ascontiguousarray(x[b].T).astype(bf),
                "wq": np.ascontiguousarray(np.asarray(Wq)[:, sl]).astype(bf),
                "wk": np.ascontiguousarray(np.asarray(Wk)[:, sl]).astype(bf),
                "wv": np.ascontiguousarray(np.asarray(Wv)[:, sl]).astype(bf),
                "wo": np.ascontiguousarray(np.asarray(Wo)[sl, :]).astype(bf),
            }
        )
    global _last_in_maps
    _last_in_maps = in_maps
    res = run_bass_kernel_spmd(
        _get_nc(), in_maps, core_ids=list(range(NCORES)), trace=False
    )
    out = np.zeros((B, N, D), dtype=np.float32)
    for c in range(NCORES):
        out[c // 4] += res.results[c]["y"]
    return out



# revision 7
# speedup vs baseline: 1.1310x; 1.1310x over previous
"""Multi-head causal attention (B=2, N=2048, D=1024, H=16) on 8 TRN2 NeuronCores.

Sharding: data-parallel over batch (2) x tensor-parallel over head groups (4),
so each core handles one batch element and 4 heads (256 of the 1024 hidden
channels). Wq/Wk/Wv are column-sharded, Wo row-sharded; each core emits a
partial output [2048, 1024] (bf16) that the host sums over the 4 head groups.

Per-core dataflow (matmuls bf16 with fp32 PSUM accumulation), fully
software-pipelined so the PE never idles (which would drop it out of the
2.4 GHz p-state):

  Q^T/K^T/V projections and the output projection are *interleaved into the
  attention loop* as filler units between S/PV iterations -- chunk qc's
  attention runs while chunk qc+1's Q/K and upcoming V seq-tiles project and
  chunk qc-1's output tiles drain.

  S^T[k,q] per k-tile: two K=64 matmuls at PE row groups h0/h64 (they execute
  concurrently on the PE array), trimmed at the causal diagonal (a diagonal
  k-tile with offset di only computes q-columns [128*di, 512)).

  exp on the Scalar engine covers both heads' trimmed regions in one strided
  ACTIVATE; causal masking multiplies only the [128,128] boundary subtile by
  a single shared triangular mask.

  PV (U += V_aug^T expS) lags S by 2 k-tiles; V is stored per seq-tile with
  ones/zeros scaffolding so the softmax denominator accumulates as an extra
  U row (even head: partition 64, odd head: partition 0).

  Normalize per (chunk, head-pair): the two denominator rows are copied to
  SBUF (bf16), partition-broadcast with two K=1 rank-1 matmuls into a PSUM
  tile, reciprocal'd in one DVE op, then ctx^T = U * (1/r).  No DRAM bounce.

  Y = ctx^T^T Wo emitted per seq-tile half as filler during later chunks,
  cast to bf16 and DMA'd out (host sums partials in fp32).
"""

import sys

sys.path.insert(0, "/opt/trn_rl_repo")

from collections import deque

import numpy as np
import ml_dtypes

import concourse.bass as bass
import concourse.bacc as bacc
import concourse.mybir as mybir
from concourse.tile import TileContext
from concourse.bass_utils import run_bass_kernel_spmd

BF16 = mybir.dt.bfloat16
F32 = mybir.dt.float32

B, N, D, H = 2, 2048, 1024, 16
HD = 64          # head dim
HPC = 4          # heads per core
DH = HPC * HD    # 256 hidden channels per core
NCORES = 8
KT = D // 128    # 8 contraction tiles over D
ST = N // 128    # 16 seq tiles
QC = N // 512    # 4 q-chunks of 512
LAG = 2          # PV lags S by this many k-tiles

# v_sb per-seq-tile column layout: for each head pair, an "even" block
# [V(64) | ones(1)] (matmul M=65 -> U partitions 0..64, denom at 64) and an
# "odd" block [ones(1) | zeros(63) | V(64)] (M=128 -> U partitions 64..127
# hold data, denom at partition 0, zeros keep partitions 1..63 inert).
V_BLK = {0: (0, 65), 1: (65, 193), 2: (193, 258), 3: (258, 386)}
V_COLS = 386


def _build_nc() -> bass.Bass:
    nc = bacc.Bacc()
    xT = nc.declare_dram_parameter("xT", [D, N], BF16, isOutput=False)
    wq = nc.declare_dram_parameter("wq", [D, DH], BF16, isOutput=False)
    wk = nc.declare_dram_parameter("wk", [D, DH], BF16, isOutput=False)
    wv = nc.declare_dram_parameter("wv", [D, DH], BF16, isOutput=False)
    wo = nc.declare_dram_parameter("wo", [DH, D], BF16, isOutput=False)
    y = nc.declare_dram_parameter("y", [N, D], BF16, isOutput=True)

    xT_r = xT.rearrange("(t p) n -> t p n", p=128)
    wq_r = wq.rearrange("(t p) m -> t p m", p=128)
    wk_r = wk.rearrange("(t p) m -> t p m", p=128)
    wv_r = wv.rearrange("(t p) m -> t p m", p=128)
    wo_r = wo.rearrange("(t p) m -> t p m", p=128)
    y_r = y.rearrange("(t p) m -> t p m", p=128)

    with TileContext(nc) as tc:
        with (
            tc.tile_pool(name="const", bufs=1) as cpool,
            tc.tile_pool(name="io", bufs=3) as io_pool,
            tc.tile_pool(name="exps", bufs=4) as exp_pool,
            tc.tile_pool(name="small", bufs=2) as small_pool,
            tc.tile_pool(name="ps_s", bufs=2, space="PSUM") as ps_s_pool,
            tc.tile_pool(name="ps_u", bufs=2, space="PSUM") as ps_u_pool,
            tc.tile_pool(name="ps_m", bufs=2, space="PSUM") as ps_m_pool,
        ):
            xT_sb = cpool.tile([128, KT, N], BF16)
            wq_sb = cpool.tile([128, KT, DH], BF16)
            wk_sb = cpool.tile([128, KT, DH], BF16)
            wv_sb = cpool.tile([128, KT, DH], BF16)
            wo_sb = cpool.tile([128, 2, D], BF16)
            qT_sb = cpool.tile([128, 2, N], BF16)
            kT_sb = cpool.tile([128, 2, N], BF16)
            v_sb = cpool.tile([128, ST, V_COLS], BF16)
            ctxT_sb = cpool.tile([128, 2, N], BF16)
            mask_sb = cpool.tile([128, 128], BF16)
            ones_sb = cpool.tile([128, 64], BF16)

            # Shared [128,128] causal boundary mask: keep (1.0) where q >= k.
            # Emitted before the xT DMAs so the gpsimd queue produces it early.
            nc.vector.memset(mask_sb, 1.0)
            nc.gpsimd.affine_select(
                out=mask_sb,
                in_=mask_sb,
                compare_op=mybir.AluOpType.is_ge,
                fill=0.0,
                base=0,
                pattern=[[1, 128]],
                channel_multiplier=-1,
            )
            nc.vector.memset(ones_sb, 1.0)
            # ones / zeros scaffolding of the V blocks (all seq tiles at once)
            nc.vector.memset(v_sb[:, :, 66:129], 0.0)
            nc.vector.memset(v_sb[:, :, 259:322], 0.0)
            for col in (64, 65, 257, 258):
                nc.vector.memset(v_sb[:, :, col : col + 1], 1.0)

            # Weights stream on the sync DMA queue; xT streams (in q-chunk
            # quarters, so chunk-0 data lands first) on the gpsimd queue.
            # The two queues run concurrently.
            for t in range(KT):
                nc.sync.dma_start(out=wq_sb[:, t, :], in_=wq_r[t])
            for t in range(KT):
                nc.sync.dma_start(out=wk_sb[:, t, :], in_=wk_r[t])
            for t in range(KT):
                nc.sync.dma_start(out=wv_sb[:, t, :], in_=wv_r[t])
            for t in range(2):
                nc.sync.dma_start(out=wo_sb[:, t, :], in_=wo_r[t])
            for qc in range(QC):
                for t in range(KT):
                    nc.gpsimd.dma_start(
                        out=xT_sb[:, t, 512 * qc : 512 * (qc + 1)],
                        in_=xT_r[t][:, 512 * qc : 512 * (qc + 1)],
                    )

            # ---------------- filler units ----------------
            def emit_q(qc2, mt, w_sb=wq_sb, dst=None):
                dst = qT_sb if dst is None else dst
                ps = ps_m_pool.tile([128, 512], F32, tag="misc", name="mps")
                for kt in range(KT):
                    nc.tensor.matmul(
                        ps,
                        lhsT=w_sb[:, kt, 128 * mt : 128 * (mt + 1)],
                        rhs=xT_sb[:, kt, 512 * qc2 : 512 * (qc2 + 1)],
                        start=(kt == 0),
                        stop=(kt == KT - 1),
                    )
                nc.vector.tensor_copy(
                    dst[:, mt, 512 * qc2 : 512 * (qc2 + 1)], ps
                )

            def emit_k(kc, mt):
                emit_q(kc, mt, w_sb=wk_sb, dst=kT_sb)

            def emit_v(st):
                ps = ps_m_pool.tile([128, 512], F32, tag="misc", name="mps")
                psv = ps[:, 0:DH]
                for kt in range(KT):
                    nc.tensor.matmul(
                        psv,
                        lhsT=xT_sb[:, kt, 128 * st : 128 * (st + 1)],
                        rhs=wv_sb[:, kt, :],
                        start=(kt == 0),
                        stop=(kt == KT - 1),
                    )
                # even heads 0,2 -> v_sb offsets 0,193; odd heads 1,3 -> 129,322
                ev = bass.AP(
                    tensor=v_sb.tensor,
                    offset=v_sb[:, st, 0:1].offset,
                    ap=[v_sb.ap[0], [193, 2], [1, HD]],
                )
                od = bass.AP(
                    tensor=v_sb.tensor,
                    offset=v_sb[:, st, 129:130].offset,
                    ap=[v_sb.ap[0], [193, 2], [1, HD]],
                )
                in_ev = bass.AP(
                    tensor=ps.tensor,
                    offset=ps[:, 0:1].offset,
                    ap=[ps.ap[0], [2 * HD, 2], [1, HD]],
                )
                in_od = bass.AP(
                    tensor=ps.tensor,
                    offset=ps[:, HD : HD + 1].offset,
                    ap=[ps.ap[0], [2 * HD, 2], [1, HD]],
                )
                nc.vector.tensor_copy(ev, in_ev)
                nc.vector.tensor_copy(od, in_od)

            def emit_y(st, half):
                ps = ps_m_pool.tile([128, 512], F32, tag="misc", name="mps")
                for kt2 in range(2):
                    nc.tensor.matmul(
                        ps,
                        lhsT=ctxT_sb[:, kt2, 128 * st : 128 * (st + 1)],
                        rhs=wo_sb[:, kt2, 512 * half : 512 * (half + 1)],
                        start=(kt2 == 0),
                        stop=(kt2 == 1),
                    )
                ysb = io_pool.tile([128, 512], BF16)
                nc.vector.tensor_copy(ysb, ps)
                nc.gpsimd.dma_start(
                    out=y_r[st][:, 512 * half : 512 * (half + 1)], in_=ysb
                )

            def emit(unit):
                if unit is None:
                    return
                kind = unit[0]
                if kind == "q":
                    emit_q(unit[1], unit[2])
                elif kind == "k":
                    emit_k(unit[1], unit[2])
                elif kind == "v":
                    emit_v(unit[1])
                elif kind == "y":
                    emit_y(unit[1], unit[2])

            # ---------------- normalize chain ----------------
            # Runs at the start of the *following* stream, spread over its
            # first three iterations so no engine queue head-of-line blocks:
            # the U banks are freed once step3's multiplies read them, just
            # before the next stream's first PV (LAG=2) needs them.
            def make_norm_steps(qc, mt, ue, uo):
                rtmp = small_pool.tile([128, 1024], BF16, tag="rtmp")
                rb = small_pool.tile([128, 512], F32, tag="rb")

                def s1():
                    # denominator rows -> SBUF (bf16)
                    nc.vector.tensor_copy(rtmp[64:65, 0:512], ue[64:65, :])
                    nc.vector.tensor_copy(rtmp[0:1, 512:1024], uo[0:1, :])

                def s2():
                    # partition-broadcast via two K=1 rank-1 matmuls, then 1/r
                    pb = ps_m_pool.tile([128, 512], F32, tag="misc", name="pb")
                    nc.tensor.matmul(
                        pb[0:64, :],
                        lhsT=ones_sb[64:65, :],
                        rhs=rtmp[64:65, 0:512],
                        start=True,
                        stop=True,
                    )
                    nc.tensor.matmul(
                        pb[64:128, :],
                        lhsT=ones_sb[0:1, :],
                        rhs=rtmp[0:1, 512:1024],
                        start=True,
                        stop=True,
                    )
                    nc.vector.reciprocal_approx_fast(out=rb, in_=pb)

                def s3():
                    nc.vector.tensor_mul(
                        ctxT_sb[0:64, mt, 512 * qc : 512 * (qc + 1)],
                        ue[0:64, :],
                        rb[0:64, :],
                    )
                    nc.vector.tensor_mul(
                        ctxT_sb[64:128, mt, 512 * qc : 512 * (qc + 1)],
                        uo[64:128, :],
                        rb[64:128, :],
                    )

                return [(0, s1), (1, s2), (2, s3)]

            # ---------------- schedules ----------------
            # preamble: minimal inputs for chunk-0/mt-0 attention
            emit_q(0, 0)
            emit_k(0, 0)
            emit_v(0)
            emit_v(1)

            # one filler per attention iteration (none at iteration 0 of
            # streams with a pending normalize chain)
            fifo = deque(
                [
                    # c0 m0 (iters 0-3, 4 slots)
                    ("v", 2), ("v", 3), ("q", 0, 1), ("k", 0, 1),
                    # c0 m1 (iters 1-3, 3)
                    ("v", 4), ("v", 5), ("q", 1, 0),
                    # c1 m0 (7)
                    ("k", 1, 0), ("q", 1, 1), ("k", 1, 1), ("v", 6),
                    ("v", 7), ("y", 0, 0), ("y", 0, 1),
                    # c1 m1 (7)
                    ("y", 1, 0), ("y", 1, 1), ("y", 2, 0), ("y", 2, 1),
                    ("q", 2, 0), ("k", 2, 0), ("v", 8),
                    # c2 m0 (11) -- chunk-1 Y units only from iter 3 on (the
                    # c1/m1 normalize's last step lands at iter 2)
                    ("q", 2, 1), ("k", 2, 1), ("v", 9),
                    ("y", 3, 0), ("y", 3, 1), ("v", 10), ("v", 11),
                    ("y", 4, 0), ("y", 4, 1), ("y", 5, 0), ("y", 5, 1),
                    # c2 m1 (11)
                    ("y", 6, 0), ("y", 6, 1), ("y", 7, 0), ("y", 7, 1),
                    ("q", 3, 0), ("k", 3, 0), ("v", 12), ("v", 13),
                    None, None, None,
                    # c3 m0 (15)
                    ("q", 3, 1), ("k", 3, 1), ("v", 14), ("v", 15),
                    ("y", 8, 0), ("y", 8, 1), ("y", 9, 0), ("y", 9, 1),
                    ("y", 10, 0), ("y", 10, 1),
                    None, None, None, None, None,
                    # c3 m1 (15): ACT-gated phase, nothing to fill
                ]
                + [None] * 15
                # held back for the tail (interleave with the last normalize)
                + [("y", 11, 0), ("y", 11, 1)]
            )

            # ---------------- attention streams ----------------
            pending = []  # normalize steps due in the current stream
            streams = [(qc, mt) for qc in range(QC) for mt in range(2)]
            for si, (qc, mt) in enumerate(streams):
                nkt = 4 * (qc + 1)
                pv_q = []  # (ex tile, kt, off)

                ue = ps_u_pool.tile([128, 512], F32, tag="u", name="ue")
                uo = ps_u_pool.tile([128, 512], F32, tag="u", name="uo")
                uu = {0: ue, 1: uo}

                def emit_pv(ex_prev, kt_prev, off_prev):
                    for parity in (0, 1):
                        head = 2 * mt + parity
                        blo, bhi = V_BLK[head]
                        ex_ap = bass.AP(
                            tensor=ex_prev.tensor,
                            offset=ex_prev[
                                :,
                                512 * parity + off_prev : 512 * parity
                                + off_prev
                                + 1,
                            ].offset,
                            ap=[ex_prev.ap[0], [1, 512 - off_prev]],
                        )
                        nc.tensor.matmul(
                            uu[parity][0 : bhi - blo, off_prev:512],
                            lhsT=v_sb[:, kt_prev, blo:bhi],
                            rhs=ex_ap,
                            start=(kt_prev == 0),
                            stop=(kt_prev == nkt - 1),
                            skip_group_check=True,
                        )

                for it in range(nkt):
                    kt = it
                    # filler unit first (its DVE drain lands ahead of this
                    # iteration's mask-mul in the DVE queue); no filler at
                    # iteration 0 when a normalize chain is pending
                    if fifo and (it > 0 or si == 0):
                        emit(fifo.popleft())

                    di = kt - 4 * qc
                    off = 128 * di if di >= 0 else 0

                    # S^T for both heads of the pair; the two K=64 matmuls
                    # occupy PE row strips h0/h64 and run concurrently
                    ps_s = ps_s_pool.tile([128, 1024], F32, tag="s", name="s")
                    for parity in (0, 1):
                        pofs = 64 * parity
                        nc.tensor.matmul(
                            ps_s[:, 512 * parity + off : 512 * (parity + 1)],
                            lhsT=kT_sb[
                                pofs : pofs + 64, mt, 128 * kt : 128 * (kt + 1)
                            ],
                            rhs=qT_sb[
                                pofs : pofs + 64,
                                mt,
                                512 * qc + off : 512 * (qc + 1),
                            ],
                            start=True,
                            stop=True,
                        )

                    # previous stream's normalize steps slot in right after
                    # the S pair (their PE rank-1 matmuls are tiny)
                    while pending and it >= pending[0][0]:
                        pending.pop(0)[1]()

                    ex = exp_pool.tile([128, 1024], BF16)
                    w = 512 - off
                    src_ap = bass.AP(
                        tensor=ps_s.tensor,
                        offset=ps_s[:, off : off + 1].offset,
                        ap=[ps_s.ap[0], [512, 2], [1, w]],
                    )
                    dst_ap = bass.AP(
                        tensor=ex.tensor,
                        offset=ex[:, off : off + 1].offset,
                        ap=[ex.ap[0], [512, 2], [1, w]],
                    )
                    nc.scalar.activation(
                        dst_ap,
                        src_ap,
                        mybir.ActivationFunctionType.Exp,
                        scale=1.0 / np.sqrt(HD),
                    )
                    if di >= 0:
                        # mask only the [128,128] boundary subtile (both
                        # parities in one strided multiply)
                        exm = bass.AP(
                            tensor=ex.tensor,
                            offset=ex[:, off : off + 1].offset,
                            ap=[ex.ap[0], [512, 2], [1, 128]],
                        )
                        mk = bass.AP(
                            tensor=mask_sb.tensor,
                            offset=mask_sb[:, 0:1].offset,
                            ap=[mask_sb.ap[0], [0, 2], [1, 128]],
                        )
                        nc.vector.tensor_mul(exm, exm, mk)

                    pv_q.append((ex, kt, off))
                    if it >= LAG:
                        emit_pv(*pv_q[it - LAG])

                # drain the PV pipeline
                for j in range(max(0, nkt - LAG), nkt):
                    emit_pv(*pv_q[j])
                while pending:  # safety: flush any unplaced steps
                    pending.pop(0)[1]()
                pending = make_norm_steps(qc, mt, ue, uo)

            # ---------------- tail ----------------
            # c3/m1 normalize interleaved with the held-back Y units, then
            # the final chunk's output projection
            pending[0][1]()
            emit(fifo.popleft())
            pending[1][1]()
            emit(fifo.popleft())
            pending[2][1]()
            for st in range(12, 16):
                for half in range(2):
                    emit_y(st, half)

    nc.finalize()
    return nc


_NC = None


def _get_nc():
    global _NC
    if _NC is None:
        _NC = _build_nc()
    return _NC


def kernel(x, Wq, Wk, Wv, Wo):
    x = np.asarray(x, dtype=np.float32)
    bf = ml_dtypes.bfloat16
    in_maps = []
    for c in range(NCORES):
        b, g = divmod(c, 4)
        sl = slice(g * DH, (g + 1) * DH)
        in_maps.append(
            {
                "xT": np.ascontiguousarray(x[b].T).astype(bf),
                "wq": np.ascontiguousarray(np.asarray(Wq)[:, sl]).astype(bf),
                "wk": np.ascontiguousarray(np.asarray(Wk)[:, sl]).astype(bf),
                "wv": np.ascontiguousarray(np.asarray(Wv)[:, sl]).astype(bf),
                "wo": np.ascontiguousarray(np.asarray(Wo)[sl, :]).astype(bf),
            }
        )
    global _last_in_maps
    _last_in_maps = in_maps
    res = run_bass_kernel_spmd(
        _get_nc(), in_maps, core_ids=list(range(NCORES)), trace=False
    )
    out = np.zeros((B, N, D), dtype=np.float32)
    for c in range(NCORES):
        out[c // 4] += res.results[c]["y"].astype(np.float32)
    return out
